# revision 27
# baseline (speedup 1.0000x reference)
# MoE (8 experts, top-2) Trainium2 kernel — expert-parallel sharding.
#
# Strategy (per sharding hint): shard the stacked expert weights along the
# expert axis across the 8 cores. The router gate is tiny (0.05% of FLOPs);
# it runs on host (bitwise-identical jnp ops to the reference) and decides
# the token dispatch ("all-to-all") which here is folded into the host->device
# sharding step: core e receives the (capacity-padded, feature-major) set of
# tokens routed to expert e and runs that expert's dense 2-layer FFN.
# The weighted top-2 combine + load-balancing loss are host-side (again
# bitwise-identical to the reference formulas).
#
# Device kernel (per core), fully fused per 512-token chunk:
#   layer 1: h^T = relu(W1^T x^T + b1)  (feature-major, h stays in SBUF)
#   layer 2: y^T = W2^T h^T             (feature-major, streamed to HBM)
# Matmuls run in float32r (full PE rate, tf32-like precision, ~2e-4 end to
# end); contraction dims live on partitions; W1/W2 are SBUF-resident with
# their loads interleaved into chunk-0's consumption order so the kernel
# prologue is HBM-bandwidth-bound rather than serialized; PSUM is juggled
# as one 8-bank pool (k-outer accumulation groups).

import sys

sys.path.insert(0, "/opt/trn_rl_repo")

import numpy as np

NUM_EXPERTS = 8
TOP_K = 2
LB_LOSS_WEIGHT = 0.01
B, S, D, DFF = 4, 2048, 2048, 1024
T = B * S  # 8192 tokens

# Per-expert token capacity. Seed-0 counts are [2093 1996 2070 2049 1999
# 2093 2097 1987] (max 2097); anything beyond CAP is computed by the tested
# host-side fallback, so a count shift only costs host time, never
# correctness.
CAP = 2112
CHUNKS = [512, 512, 512, 320, 256]  # keep every chunk >=256: fp32r matmul
assert sum(CHUNKS) == CAP           # drops to 1/4 rate below 256 columns

_RUNNER = {}


def _build_nc(repeat=1):
    from concourse import bacc
    import concourse.tile as tile
    import concourse.mybir as mybir

    f32 = mybir.dt.float32
    f32r = mybir.dt.float32r  # full-rate fp32 matmul (tf32-like, ~1.5e-4)
    Relu = mybir.ActivationFunctionType.Relu
    KD = D // 128  # 16 k-tiles over D
    MF = DFF // 128  # 8 m-tiles over DFF
    ND = D // 128  # 16 out-tiles over D

    nc = bacc.Bacc(None)
    xT = nc.dram_tensor("xT", (D, CAP), f32r, kind="ExternalInput")
    w1 = nc.dram_tensor("w1", (D, DFF), f32r, kind="ExternalInput")
    w2 = nc.dram_tensor("w2", (DFF, D), f32r, kind="ExternalInput")
    b1 = nc.dram_tensor("b1", (128, MF), f32, kind="ExternalInput")
    yT = nc.dram_tensor("yT", (D, CAP), f32, kind="ExternalOutput")

    xTv = xT.rearrange("(ko p) n -> p ko n", p=128)
    yTv = yT.rearrange("(mo p) n -> p mo n", p=128)

    w1v = w1.rearrange("(ko p) f -> p ko f", p=128)
    w2v = w2.rearrange("(ko p) f -> p ko f", p=128)

    with tile.TileContext(nc) as tc:
        with tc.tile_pool(name="wres", bufs=1) as wres, tc.tile_pool(
            name="xtp", bufs=1
        ) as xtp, tc.tile_pool(name="hp", bufs=1) as hp, tc.tile_pool(
            name="ysp", bufs=3
        ) as ysp, tc.tile_pool(name="psp", bufs=8, space="PSUM") as psp:
            # resident weights; per-k-slice DMAs, interleaved with chunk-0's
            # x slices in layer-1 consumption order so the PE starts within a
            # few us instead of waiting for the full 16.8MB weight load
            b1sb = wres.tile([128, MF], f32)
            nc.sync.dma_start(b1sb[:], b1[:])
            w1sb = wres.tile([128, KD, DFF], f32r)
            w2sb = wres.tile([128, MF, D], f32r)
            xt0 = xtp.tile([128, KD, 512], f32r, tag="xt")
            ch0 = CHUNKS[0]
            for k in range(KD):
                nc.sync.dma_start(xt0[:, k, :ch0], xTv[:, k, 0:ch0])
                nc.sync.dma_start(w1sb[:, k, :], w1v[:, k, :])
            for k in range(MF):
                nc.sync.dma_start(w2sb[:, k, :], w2v[:, k, :])

            for rep in range(repeat):
              off = 0
              for c, ch in enumerate(CHUNKS):
                if c == 0 and rep == 0:
                    xt = xt0
                else:
                    # prefetched under the previous chunk's layer 2: one big
                    # batched DMA is cheaper than 16 slice DMAs
                    xt = xtp.tile([128, KD, 512], f32r, tag="xt")
                    nc.sync.dma_start(xt[:, :, :ch], xTv[:, :, off : off + ch])
                # layer 1: h[:, m, :] = relu(W1^T x^T + b1), DFF-major.
                # Chunk 0 runs k-outer with all 8 m-accumulators live (8 PSUM
                # banks) so the PE consumes each streamed-in w1/x k-slice the
                # moment it lands; later chunks run m-outer so the ACT relu
                # drains pipeline per-m with no group-end burst.
                h = hp.tile([128, MF, 512], f32r, tag="h")
                if c == 0 and rep == 0:
                    pss = [
                        psp.tile([128, 512], f32, tag="ps", name=f"psA{m}")
                        for m in range(MF)
                    ]
                    for k in range(KD):
                        for m in range(MF):
                            nc.tensor.matmul(
                                pss[m][:, :ch],
                                w1sb[:, k, m * 128 : (m + 1) * 128],
                                xt[:, k, :ch],
                                start=(k == 0),
                                stop=(k == KD - 1),
                            )
                    for m in range(MF):
                        nc.scalar.activation(
                            h[:, m, :ch], pss[m][:, :ch], Relu, bias=b1sb[:, m : m + 1]
                        )
                else:
                    for m in range(MF):
                        ps = psp.tile([128, 512], f32, tag="ps", name=f"psA{m}")
                        for k in range(KD):
                            nc.tensor.matmul(
                                ps[:, :ch],
                                w1sb[:, k, m * 128 : (m + 1) * 128],
                                xt[:, k, :ch],
                                start=(k == 0),
                                stop=(k == KD - 1),
                            )
                        nc.scalar.activation(
                            h[:, m, :ch], ps[:, :ch], Relu, bias=b1sb[:, m : m + 1]
                        )
                # layer 2: y^T[md] = W2^T h^T, D-major; md-groups with k-outer
                # (k here is h's m-axis, so low k is ready first). Chunk 0
                # uses groups of 8 (first group is paced by the w2 stream
                # anyway); steady chunks use groups of 4 so two groups are in
                # flight and the copy drain overlaps the next group's matmuls.
                gsz = 8 if (c == 0 and rep == 0) else 4
                for g in range(ND // gsz):
                    pss = [
                        psp.tile([128, 512], f32, tag="ps", name=f"psB{i}")
                        for i in range(gsz)
                    ]
                    for k in range(MF):
                        for i in range(gsz):
                            md = g * gsz + i
                            nc.tensor.matmul(
                                pss[i][:, :ch],
                                w2sb[:, k, md * 128 : (md + 1) * 128],
                                h[:, k, :ch],
                                start=(k == 0),
                                stop=(k == MF - 1),
                            )
                    for b in range(gsz // 4):
                        ys = ysp.tile([128, 4, 512], f32, tag="ys")
                        for i in range(4):
                            nc.vector.tensor_copy(
                                ys[:, i, :ch], pss[b * 4 + i][:, :ch]
                            )
                        md0 = g * gsz + b * 4
                        nc.sync.dma_start(
                            yTv[:, md0 : md0 + 4, off : off + ch],
                            ys[:, :, :ch],
                        )
                off += ch

    nc.compile()
    return nc


def _get_runner():
    """Build (once) the 8-core jitted PJRT callable: list of 8 per-core input
    dicts -> list of 8 output dicts. Mirrors bass2jax.run_bass_via_pjrt's
    multi-core path but caches the jitted fn so repeated calls don't
    recompile, and skips donation (kernel overwrites every output element)."""
    if _RUNNER:
        return _RUNNER["fn"]
    import jax
    import concourse.mybir as mybir
    from concourse import bass2jax
    from jax.experimental.shard_map import shard_map
    from jax.sharding import Mesh, PartitionSpec

    nc = _build_nc()
    bass2jax.install_neuronx_cc_hook()

    partition_name = nc.partition_id_tensor.name if nc.partition_id_tensor else None
    in_names, out_names, out_avals = [], [], []
    for alloc in nc.m.functions[0].allocations:
        if not isinstance(alloc, mybir.MemoryLocationSet):
            continue
        name = alloc.memorylocations[0].name
        if alloc.kind == "ExternalInput":
            if name != partition_name:
                in_names.append(name)
        elif alloc.kind == "ExternalOutput":
            out_names.append(name)
            out_avals.append(
                jax.core.ShapedArray(tuple(alloc.tensor_shape), mybir.dt.np(alloc.dtype))
            )
    n_params = len(in_names)
    zero_outs = [np.zeros(a.shape, a.dtype) for a in out_avals]
    all_in_names = in_names + out_names
    if partition_name is not None:
        all_in_names.append(partition_name)

    def _body(*args):
        operands = list(args)
        if partition_name is not None:
            operands.append(bass2jax.partition_id_tensor())
        outs = bass2jax._bass_exec_p.bind(
            *operands,
            out_avals=tuple(out_avals),
            in_names=tuple(all_in_names),
            out_names=tuple(out_names),
            lowering_input_output_aliases=(),
            sim_require_finite=True,
            sim_require_nnan=True,
            nc=nc,
        )
        return tuple(outs)

    devices = jax.devices()[:NUM_EXPERTS]
    mesh = Mesh(np.asarray(devices), ("core",))
    n_out = len(out_names)
    sharded = jax.jit(
        shard_map(
            _body,
            mesh=mesh,
            in_specs=(PartitionSpec("core"),) * (n_params + n_out),
            out_specs=(PartitionSpec("core"),) * n_out,
            check_rep=False,
        ),
        keep_unused=True,
    )
    zeros_concat = [
        jax.device_put(
            np.zeros((NUM_EXPERTS * z.shape[0], *z.shape[1:]), z.dtype),
            jax.sharding.NamedSharding(mesh, PartitionSpec("core")),
        )
        for z in zero_outs
    ]

    def run(in_maps):
        concat_in = [
            np.concatenate([m[name] for m in in_maps], axis=0) for name in in_names
        ]
        out = sharded(*concat_in, *zeros_concat)
        return [
            {
                name: np.asarray(out[i]).reshape(
                    NUM_EXPERTS, *out_avals[i].shape
                )[c]
                for i, name in enumerate(out_names)
            }
            for c in range(NUM_EXPERTS)
        ]

    _RUNNER["fn"] = run
    _RUNNER["sharded"] = sharded
    _RUNNER["meta"] = (in_names, out_names, out_avals, zeros_concat, mesh)
    return run


def _route_host(x, Wg):
    """Router + lb-loss with the reference's exact jnp ops on CPU so the
    top-k selection and the returned loss are bitwise-identical to it."""
    import jax
    import jax.numpy as jnp

    cpu = jax.devices("cpu")[0]
    with jax.default_device(cpu):
        xj = jnp.asarray(x)
        logits = jnp.einsum("bsd,de->bse", xj, jnp.asarray(Wg))
        top_vals, top_idx = jax.lax.top_k(logits, TOP_K)
        expert_weights = jax.nn.softmax(top_vals, axis=-1)
        probs = jax.nn.softmax(logits, axis=-1)
        avg_probs = probs.mean(axis=(0, 1))
        top1 = jnp.argmax(logits, axis=-1).reshape(-1)
        freq = jnp.sum(jax.nn.one_hot(top1, NUM_EXPERTS, dtype=logits.dtype), axis=0)
        freq = freq / freq.sum()
        lb_loss = NUM_EXPERTS * jnp.sum(freq * avg_probs)
        lb = np.asarray(LB_LOSS_WEIGHT * lb_loss)
    return (
        np.asarray(top_idx).reshape(T, TOP_K),
        np.asarray(expert_weights).reshape(T, TOP_K).astype(np.float32),
        lb,
    )


def _ffn_host(xe, W1e, b1e, W2e):
    """Host fallback for capacity overflow (never hit for the shipped seed)."""
    h = np.maximum(xe @ W1e + b1e, 0.0)
    return h @ W2e


def kernel(x, Wg, W1, b1, W2, b2):
    x = np.ascontiguousarray(np.asarray(x, dtype=np.float32))
    Wg = np.asarray(Wg, dtype=np.float32)
    W1 = np.asarray(W1, dtype=np.float32)
    b1 = np.asarray(b1, dtype=np.float32)
    W2 = np.asarray(W2, dtype=np.float32)
    b2 = np.asarray(b2, dtype=np.float32)

    top_idx, wts, lb = _route_host(x, Wg)
    x_flat = x.reshape(T, D)

    # dispatch: token index lists + position-in-expert for each (token, slot)
    idx_lists = []
    pos_in_expert = np.zeros((T, TOP_K), dtype=np.int64)
    for e in range(NUM_EXPERTS):
        sel0 = top_idx[:, 0] == e
        sel1 = top_idx[:, 1] == e
        idx_e = np.nonzero(sel0 | sel1)[0]
        idx_lists.append(idx_e)
        slot = np.where(sel0[idx_e], 0, 1)
        pos_in_expert[idx_e, slot] = np.arange(len(idx_e))

    run = _get_runner()
    in_maps = []
    for e in range(NUM_EXPERTS):
        idx_e = idx_lists[e][:CAP]
        xTe = np.zeros((D, CAP), dtype=np.float32)
        xTe[:, : len(idx_e)] = x_flat[idx_e].T
        in_maps.append(
            {
                "xT": xTe,
                "w1": np.ascontiguousarray(W1[e]),
                "w2": np.ascontiguousarray(W2[e]),
                "b1": np.ascontiguousarray(b1[e].reshape(DFF // 128, 128).T),
            }
        )
    res = run(in_maps)
    Y = np.stack([res[e]["yT"].T for e in range(NUM_EXPERTS)])  # (E, CAP, D)

    # overflow fallback: any token beyond CAP for its expert computed on host
    over = [e for e in range(NUM_EXPERTS) if len(idx_lists[e]) > CAP]
    Y_over = {}
    for e in over:
        extra = idx_lists[e][CAP:]
        Y_over[e] = (_ffn_host(x_flat[extra], W1[e], b1[e], W2[e]), extra)

    out = np.zeros((T, D), dtype=np.float32)
    for s in range(TOP_K):
        es = top_idx[:, s]
        ps = pos_in_expert[:, s]
        contrib = Y[es, np.minimum(ps, CAP - 1)]  # (T, D) gather
        if over:
            for e in over:
                ye, extra = Y_over[e]
                mask = (es == e) & (ps >= CAP)
                if mask.any():
                    sel = np.nonzero(mask)[0]
                    remap = {t: i for i, t in enumerate(extra)}
                    contrib[sel] = ye[[remap[t] for t in sel]]
        out += wts[:, s : s + 1] * (contrib + b2[es])

    return out.reshape(B, S, D), lb.astype(np.float32)


# revision 28
# speedup vs baseline: 1.0073x; 1.0073x over previous
# MoE (8 experts, top-2) Trainium2 kernel — expert-parallel sharding.
#
# Strategy (per sharding hint): shard the stacked expert weights along the
# expert axis across the 8 cores. The router gate is tiny (0.05% of FLOPs);
# it runs on host (bitwise-identical jnp ops to the reference) and decides
# the token dispatch ("all-to-all") which here is folded into the host->device
# sharding step: core e receives the (capacity-padded, feature-major) set of
# tokens routed to expert e and runs that expert's dense 2-layer FFN.
# The weighted top-2 combine + load-balancing loss are host-side (again
# bitwise-identical to the reference formulas).
#
# Device kernel (per core), fully fused per 512-token chunk:
#   layer 1: h^T = relu(W1^T x^T + b1)  (feature-major, h stays in SBUF)
#   layer 2: y^T = W2^T h^T             (feature-major, streamed to HBM)
# Matmuls run in float32r (full PE rate, tf32-like precision, ~2e-4 end to
# end); contraction dims live on partitions; W1/W2 are SBUF-resident with
# their loads interleaved into chunk-0's consumption order so the kernel
# prologue is HBM-bandwidth-bound rather than serialized; PSUM is juggled
# as one 8-bank pool (k-outer accumulation groups).

import sys

sys.path.insert(0, "/opt/trn_rl_repo")

import numpy as np

NUM_EXPERTS = 8
TOP_K = 2
LB_LOSS_WEIGHT = 0.01
B, S, D, DFF = 4, 2048, 2048, 1024
T = B * S  # 8192 tokens

# Per-expert token capacity. Seed-0 counts are [2093 1996 2070 2049 1999
# 2093 2097 1987] (max 2097); anything beyond CAP is computed by the tested
# host-side fallback, so a count shift only costs host time, never
# correctness.
CAP = 2112
CHUNKS = [512, 512, 512, 320, 256]  # keep every chunk >=256: fp32r matmul
assert sum(CHUNKS) == CAP           # drops to 1/4 rate below 256 columns

_RUNNER = {}


def _build_nc(repeat=1):
    from concourse import bacc
    import concourse.tile as tile
    import concourse.mybir as mybir

    f32 = mybir.dt.float32
    f32r = mybir.dt.float32r  # full-rate fp32 matmul (tf32-like, ~1.5e-4)
    Relu = mybir.ActivationFunctionType.Relu
    KD = D // 128  # 16 k-tiles over D
    MF = DFF // 128  # 8 m-tiles over DFF
    ND = D // 128  # 16 out-tiles over D

    nc = bacc.Bacc(None)
    xT = nc.dram_tensor("xT", (D, CAP), f32r, kind="ExternalInput")
    w1 = nc.dram_tensor("w1", (D, DFF), f32r, kind="ExternalInput")
    w2 = nc.dram_tensor("w2", (DFF, D), f32r, kind="ExternalInput")
    b1 = nc.dram_tensor("b1", (128, MF), f32, kind="ExternalInput")
    yT = nc.dram_tensor("yT", (D, CAP), f32, kind="ExternalOutput")

    xTv = xT.rearrange("(ko p) n -> p ko n", p=128)
    yTv = yT.rearrange("(mo p) n -> p mo n", p=128)

    w1v = w1.rearrange("(ko p) f -> p ko f", p=128)
    w2v = w2.rearrange("(ko p) f -> p ko f", p=128)

    with tile.TileContext(nc) as tc:
        with tc.tile_pool(name="wres", bufs=1) as wres, tc.tile_pool(
            name="xtp", bufs=1
        ) as xtp, tc.tile_pool(name="hp", bufs=1) as hp, tc.tile_pool(
            name="ysp", bufs=3
        ) as ysp, tc.tile_pool(name="psp", bufs=8, space="PSUM") as psp:
            # resident weights; per-k-slice DMAs, interleaved with chunk-0's
            # x slices in layer-1 consumption order so the PE starts within a
            # few us instead of waiting for the full 16.8MB weight load
            b1sb = wres.tile([128, MF], f32)
            w1sb = wres.tile([128, KD, DFF], f32r)
            w2sb = wres.tile([128, MF, D], f32r)
            xt0 = xtp.tile([128, KD, 512], f32r, tag="xt")
            ch0 = CHUNKS[0]
            for k in range(KD):
                nc.sync.dma_start(xt0[:, k, :ch0], xTv[:, k, 0:ch0])
                nc.sync.dma_start(w1sb[:, k, :], w1v[:, k, :])
                if k == 0:
                    nc.sync.dma_start(b1sb[:], b1[:])
            for k in range(MF):
                nc.sync.dma_start(w2sb[:, k, :], w2v[:, k, :])

            for rep in range(repeat):
              off = 0
              for c, ch in enumerate(CHUNKS):
                if c == 0 and rep == 0:
                    xt = xt0
                else:
                    # prefetched under the previous chunk's layer 2: one big
                    # batched DMA is cheaper than 16 slice DMAs
                    xt = xtp.tile([128, KD, 512], f32r, tag="xt")
                    nc.sync.dma_start(xt[:, :, :ch], xTv[:, :, off : off + ch])
                # layer 1: h[:, m, :] = relu(W1^T x^T + b1), DFF-major.
                # Chunk 0 runs k-outer with all 8 m-accumulators live (8 PSUM
                # banks) so the PE consumes each streamed-in w1/x k-slice the
                # moment it lands; later chunks run m-outer so the ACT relu
                # drains pipeline per-m with no group-end burst.
                h = hp.tile([128, MF, 512], f32r, tag="h")
                if c == 0 and rep == 0:
                    pss = [
                        psp.tile([128, 512], f32, tag="ps", name=f"psA{m}")
                        for m in range(MF)
                    ]
                    for k in range(KD):
                        for m in range(MF):
                            nc.tensor.matmul(
                                pss[m][:, :ch],
                                w1sb[:, k, m * 128 : (m + 1) * 128],
                                xt[:, k, :ch],
                                start=(k == 0),
                                stop=(k == KD - 1),
                            )
                    for m in range(MF):
                        nc.scalar.activation(
                            h[:, m, :ch], pss[m][:, :ch], Relu, bias=b1sb[:, m : m + 1]
                        )
                else:
                    for m in range(MF):
                        ps = psp.tile([128, 512], f32, tag="ps", name=f"psA{m}")
                        for k in range(KD):
                            nc.tensor.matmul(
                                ps[:, :ch],
                                w1sb[:, k, m * 128 : (m + 1) * 128],
                                xt[:, k, :ch],
                                start=(k == 0),
                                stop=(k == KD - 1),
                            )
                        nc.scalar.activation(
                            h[:, m, :ch], ps[:, :ch], Relu, bias=b1sb[:, m : m + 1]
                        )
                # layer 2: y^T[md] = W2^T h^T, D-major; md-groups with k-outer
                # (k here is h's m-axis, so low k is ready first). Chunk 0
                # uses groups of 8 (first group is paced by the w2 stream
                # anyway); steady chunks use groups of 4 so two groups are in
                # flight and the copy drain overlaps the next group's matmuls.
                gsz = 8 if (c == 0 and rep == 0) else (2 if (c == len(CHUNKS) - 1 and rep == repeat - 1) else 4)
                for g in range(ND // gsz):
                    pss = [
                        psp.tile([128, 512], f32, tag="ps", name=f"psB{i}")
                        for i in range(gsz)
                    ]
                    for k in range(MF):
                        for i in range(gsz):
                            md = g * gsz + i
                            nc.tensor.matmul(
                                pss[i][:, :ch],
                                w2sb[:, k, md * 128 : (md + 1) * 128],
                                h[:, k, :ch],
                                start=(k == 0),
                                stop=(k == MF - 1),
                            )
                    bs = min(4, gsz)
                    for b in range(gsz // bs):
                        ys = ysp.tile([128, 4, 512], f32, tag="ys")
                        for i in range(bs):
                            nc.vector.tensor_copy(
                                ys[:, i, :ch], pss[b * bs + i][:, :ch]
                            )
                        md0 = g * gsz + b * bs
                        nc.sync.dma_start(
                            yTv[:, md0 : md0 + bs, off : off + ch],
                            ys[:, :bs, :ch],
                        )
                off += ch

    nc.compile()
    return nc


def _get_runner():
    """Build (once) the 8-core jitted PJRT callable: list of 8 per-core input
    dicts -> list of 8 output dicts. Mirrors bass2jax.run_bass_via_pjrt's
    multi-core path but caches the jitted fn so repeated calls don't
    recompile, and skips donation (kernel overwrites every output element)."""
    if _RUNNER:
        return _RUNNER["fn"]
    import jax
    import concourse.mybir as mybir
    from concourse import bass2jax
    from jax.experimental.shard_map import shard_map
    from jax.sharding import Mesh, PartitionSpec

    nc = _build_nc()
    bass2jax.install_neuronx_cc_hook()

    partition_name = nc.partition_id_tensor.name if nc.partition_id_tensor else None
    in_names, out_names, out_avals = [], [], []
    for alloc in nc.m.functions[0].allocations:
        if not isinstance(alloc, mybir.MemoryLocationSet):
            continue
        name = alloc.memorylocations[0].name
        if alloc.kind == "ExternalInput":
            if name != partition_name:
                in_names.append(name)
        elif alloc.kind == "ExternalOutput":
            out_names.append(name)
            out_avals.append(
                jax.core.ShapedArray(tuple(alloc.tensor_shape), mybir.dt.np(alloc.dtype))
            )
    n_params = len(in_names)
    zero_outs = [np.zeros(a.shape, a.dtype) for a in out_avals]
    all_in_names = in_names + out_names
    if partition_name is not None:
        all_in_names.append(partition_name)

    def _body(*args):
        operands = list(args)
        if partition_name is not None:
            operands.append(bass2jax.partition_id_tensor())
        outs = bass2jax._bass_exec_p.bind(
            *operands,
            out_avals=tuple(out_avals),
            in_names=tuple(all_in_names),
            out_names=tuple(out_names),
            lowering_input_output_aliases=(),
            sim_require_finite=True,
            sim_require_nnan=True,
            nc=nc,
        )
        return tuple(outs)

    devices = jax.devices()[:NUM_EXPERTS]
    mesh = Mesh(np.asarray(devices), ("core",))
    n_out = len(out_names)
    sharded = jax.jit(
        shard_map(
            _body,
            mesh=mesh,
            in_specs=(PartitionSpec("core"),) * (n_params + n_out),
            out_specs=(PartitionSpec("core"),) * n_out,
            check_rep=False,
        ),
        keep_unused=True,
    )
    zeros_concat = [
        jax.device_put(
            np.zeros((NUM_EXPERTS * z.shape[0], *z.shape[1:]), z.dtype),
            jax.sharding.NamedSharding(mesh, PartitionSpec("core")),
        )
        for z in zero_outs
    ]

    def run(in_maps):
        concat_in = [
            np.concatenate([m[name] for m in in_maps], axis=0) for name in in_names
        ]
        out = sharded(*concat_in, *zeros_concat)
        return [
            {
                name: np.asarray(out[i]).reshape(
                    NUM_EXPERTS, *out_avals[i].shape
                )[c]
                for i, name in enumerate(out_names)
            }
            for c in range(NUM_EXPERTS)
        ]

    _RUNNER["fn"] = run
    _RUNNER["sharded"] = sharded
    _RUNNER["meta"] = (in_names, out_names, out_avals, zeros_concat, mesh)
    return run


def _route_host(x, Wg):
    """Router + lb-loss with the reference's exact jnp ops on CPU so the
    top-k selection and the returned loss are bitwise-identical to it."""
    import jax
    import jax.numpy as jnp

    cpu = jax.devices("cpu")[0]
    with jax.default_device(cpu):
        xj = jnp.asarray(x)
        logits = jnp.einsum("bsd,de->bse", xj, jnp.asarray(Wg))
        top_vals, top_idx = jax.lax.top_k(logits, TOP_K)
        expert_weights = jax.nn.softmax(top_vals, axis=-1)
        probs = jax.nn.softmax(logits, axis=-1)
        avg_probs = probs.mean(axis=(0, 1))
        top1 = jnp.argmax(logits, axis=-1).reshape(-1)
        freq = jnp.sum(jax.nn.one_hot(top1, NUM_EXPERTS, dtype=logits.dtype), axis=0)
        freq = freq / freq.sum()
        lb_loss = NUM_EXPERTS * jnp.sum(freq * avg_probs)
        lb = np.asarray(LB_LOSS_WEIGHT * lb_loss)
    return (
        np.asarray(top_idx).reshape(T, TOP_K),
        np.asarray(expert_weights).reshape(T, TOP_K).astype(np.float32),
        lb,
    )


def _ffn_host(xe, W1e, b1e, W2e):
    """Host fallback for capacity overflow (never hit for the shipped seed)."""
    h = np.maximum(xe @ W1e + b1e, 0.0)
    return h @ W2e


def kernel(x, Wg, W1, b1, W2, b2):
    x = np.ascontiguousarray(np.asarray(x, dtype=np.float32))
    Wg = np.asarray(Wg, dtype=np.float32)
    W1 = np.asarray(W1, dtype=np.float32)
    b1 = np.asarray(b1, dtype=np.float32)
    W2 = np.asarray(W2, dtype=np.float32)
    b2 = np.asarray(b2, dtype=np.float32)

    top_idx, wts, lb = _route_host(x, Wg)
    x_flat = x.reshape(T, D)

    # dispatch: token index lists + position-in-expert for each (token, slot)
    idx_lists = []
    pos_in_expert = np.zeros((T, TOP_K), dtype=np.int64)
    for e in range(NUM_EXPERTS):
        sel0 = top_idx[:, 0] == e
        sel1 = top_idx[:, 1] == e
        idx_e = np.nonzero(sel0 | sel1)[0]
        idx_lists.append(idx_e)
        slot = np.where(sel0[idx_e], 0, 1)
        pos_in_expert[idx_e, slot] = np.arange(len(idx_e))

    run = _get_runner()
    in_maps = []
    for e in range(NUM_EXPERTS):
        idx_e = idx_lists[e][:CAP]
        xTe = np.zeros((D, CAP), dtype=np.float32)
        xTe[:, : len(idx_e)] = x_flat[idx_e].T
        in_maps.append(
            {
                "xT": xTe,
                "w1": np.ascontiguousarray(W1[e]),
                "w2": np.ascontiguousarray(W2[e]),
                "b1": np.ascontiguousarray(b1[e].reshape(DFF // 128, 128).T),
            }
        )
    res = run(in_maps)
    Y = np.stack([res[e]["yT"].T for e in range(NUM_EXPERTS)])  # (E, CAP, D)

    # overflow fallback: any token beyond CAP for its expert computed on host
    over = [e for e in range(NUM_EXPERTS) if len(idx_lists[e]) > CAP]
    Y_over = {}
    for e in over:
        extra = idx_lists[e][CAP:]
        Y_over[e] = (_ffn_host(x_flat[extra], W1[e], b1[e], W2[e]), extra)

    out = np.zeros((T, D), dtype=np.float32)
    for s in range(TOP_K):
        es = top_idx[:, s]
        ps = pos_in_expert[:, s]
        contrib = Y[es, np.minimum(ps, CAP - 1)]  # (T, D) gather
        if over:
            for e in over:
                ye, extra = Y_over[e]
                mask = (es == e) & (ps >= CAP)
                if mask.any():
                    sel = np.nonzero(mask)[0]
                    remap = {t: i for i, t in enumerate(extra)}
                    contrib[sel] = ye[[remap[t] for t in sel]]
        out += wts[:, s : s + 1] * (contrib + b2[es])

    return out.reshape(B, S, D), lb.astype(np.float32)
